# revision 15
# baseline (speedup 1.0000x reference)
"""Bahdanau attention kernel for Trainium2 (8 NeuronCores, data-parallel over batch).

Shapes (hardcoded): B=32, S=2048, H=1024, QS=1024, VS=2048.
Per core: 4 batches.

Math per batch b:
  q = query[b] @ Wq                                  # [H]
  scores[s] = sum_h v_energy[h] * tanh(q[h] + proj_key[b,s,h])
  alphas = softmax(scores masked by mask[b])         # [S]
  context = alphas @ value[b]                        # [VS]

Device strategy per core:
  - All large loads in natural layout (s on partitions, h/v on free dim).
  - q computed on PE in fp32 (queryT as lhsT), broadcast across partitions
    via a DRAM bounce + partition-stride-0 load.
  - scores: DVE add (q) -> ACT tanh -> DVE mul (v_energy broadcast) ->
    ACT Identity with free-dim accumulator. Softmax without max
    subtraction (|scores| <= ||v_energy||_1 ~ 26, exp cannot overflow),
    mask applied multiplicatively (exact zeros), cross-partition sum +
    broadcast of Z via a ones matmul on PE.
  - context: alphas-stationary PE matmul in float32r (TF32-class, ~1.5e-4
    rel; full-rate streaming vs 4x-slower fp32). value tiles are loaded as
    f32r via SWDGE cast DMAs; alphas get a tiny f32r copy.
  - A post-pass splits multi-wait instructions into chains of single-wait
    NOPs (this walrus build fits only one sync-wait per instruction).
"""

import numpy as np

import concourse.bass as bass
import concourse.tile as tile
from concourse import mybir
from concourse.bass_utils import run_bass_kernel_spmd

B, S, H, QS, VS = 32, 2048, 1024, 1024, 2048
NCORES = 8
BPC = B // NCORES  # batches per core

F32 = mybir.dt.float32
F32R = mybir.dt.float32r
I32 = mybir.dt.int32

SC = S // 128  # 16 s-chunks of 128
HC = H // 128  # 8 h-chunks (for Wq/queryT layout)
PKG = 4        # s-chunks per proj_key DMA (2MB)
VG = 4         # s-chunks per value DMA (4MB)

USE_F32R_CTX = True  # fallback to exact fp32 context matmul if False

# Only these instruction types get their excess waits split onto NOPs —
# raw-encoded (InstISA) and sync-machinery instructions are left exactly
# as Tile emitted them.
_SPLIT_TYPES = (
    "InstMatmult",
    "InstDMACopy",
    "InstActivation",
    "InstTensorCopy",
    "InstTensorTensor",
    "InstTensorReduce",
    "InstTensorScalarPtr",
    "InstMemset",
    "InstReciprocal",
    "InstLdweights",
    "InstDrain",
    "InstEventSemaphore",
    "InstNoOp",
)


def _make_wait_nop(nc, engine_type, wait):
    """Build a properly-encoded NOP via the engine API (it lands at the
    tail of the current bb), detach it, and give it the single wait
    (encoded through the proper wait_op path)."""
    import bass_rust as _br

    bi = nc.engines[engine_type].nop(nofuse=True)
    sem = _br.SemaphoreHandle(wait.ant_name or f"sem{wait.id}", wait.id)
    bi._wait_ge(sem, wait.wait_value)
    ni = bi.ins
    for fn in nc.m.functions:
        for blk in fn.blocks:
            if blk.instructions and blk.instructions[-1].name == ni.name:
                lst = list(blk.instructions)
                lst.pop()
                blk.instructions = lst
                return ni
    raise RuntimeError("freshly added nop not found at any block tail")


def _split_excess_waits(nc):
    """This walrus build fits only ONE sync-wait into most instruction
    encodings ("Too many sync wait commands" codegen errors). Move every
    wait beyond the first onto standalone same-engine NOPs inserted right
    before the instruction — the sequencer waits on each in turn, which is
    semantically identical."""
    for fn in nc.m.functions:
        for blk in fn.blocks:
            offenders = [
                inst
                for inst in blk.instructions
                if inst.sync_info is not None
                and inst.sync_info.on_wait
                and len(inst.sync_info.on_wait) > 1
                and type(inst).__name__ in _SPLIT_TYPES
            ]
            if not offenders:
                continue
            pre = {}
            for inst in offenders:
                si = inst.sync_info
                waits = list(si.on_wait)
                pre[inst.name] = [
                    _make_wait_nop(nc, inst.engine, w) for w in waits[:-1]
                ]
                inst.sync_info = mybir.SyncInfo(
                    on_wait=[waits[-1]],
                    on_update=list(si.on_update) if si.on_update else [],
                )
            out = []
            for inst in blk.instructions:
                out.extend(pre.get(inst.name, ()))
                out.append(inst)
            blk.instructions = out
    return nc


def _ap(t, offset, dims):
    return bass.AP(tensor=t, offset=offset, ap=[list(d) for d in dims])


def build_nc():
    nc = bass.Bass()

    query = nc.dram_tensor("query", [BPC, QS], F32, kind="ExternalInput")
    pk = nc.dram_tensor("proj_key", [BPC, S, H], F32, kind="ExternalInput")
    value = nc.dram_tensor("value", [BPC, S, VS], F32, kind="ExternalInput")
    mask = nc.dram_tensor("mask", [BPC, 1, S], I32, kind="ExternalInput")
    wq = nc.dram_tensor("Wq", [QS, H], F32, kind="ExternalInput")
    ve = nc.dram_tensor("v_energy", [H], F32, kind="ExternalInput")
    ctx_out = nc.dram_tensor("context", [BPC, 1, VS], F32, kind="ExternalOutput")
    al_out = nc.dram_tensor("alphas", [BPC, 1, S], F32, kind="ExternalOutput")

    vdt = F32R if USE_F32R_CTX else F32

    with tile.TileContext(nc) as tc:
        with (
            tc.tile_pool(name="consts", bufs=1) as consts,
            tc.tile_pool(name="qb", bufs=1) as qbp,
            tc.tile_pool(name="dramp", bufs=1, space="DRAM") as dramp,
        ):
            # ---- constants / prologue ----
            ve_bcast = consts.tile([128, H], F32)
            nc.gpsimd.dma_start(out=ve_bcast, in_=_ap(ve, 0, [[0, 128], [1, H]]))

            ones128 = consts.tile([128, 128], F32)
            nc.vector.memset(ones128, 1.0)

            # queryT / Wq live in their own pool, released after the q
            # computation so the streaming pools can use the SBUF space
            q_sb = consts.tile([BPC, H], F32)
            with (
                tc.tile_pool(name="wqp", bufs=1) as wqp,
                tc.tile_pool(name="ps", bufs=2, space="PSUM") as ps,
            ):
                qT = wqp.tile([128, HC, BPC], F32)
                for j in range(HC):
                    nc.gpsimd.dma_start(
                        out=qT[:, j, :],
                        in_=_ap(query, j * 128, [[1, 128], [QS, BPC]]),
                    )
                wq_t = wqp.tile([128, HC, H], F32)
                nc.sync.dma_start(
                    out=wq_t, in_=_ap(wq, 0, [[H, 128], [128 * H, HC], [1, H]])
                )

                # q = query @ Wq -> psum [BPC, 1024], two 512 halves (fp32)
                for half in range(2):
                    qp = ps.tile([128, 512], F32, tag="sm")
                    for j in range(HC):
                        nc.tensor.matmul(
                            out=qp[0:BPC, :],
                            lhsT=qT[:, j, :],
                            rhs=wq_t[:, j, half * 512 : (half + 1) * 512],
                            start=(j == 0),
                            stop=(j == HC - 1),
                        )
                    nc.vector.tensor_copy(
                        out=q_sb[:, half * 512 : (half + 1) * 512], in_=qp[0:BPC, :]
                    )

            # broadcast q[b] across 128 partitions via DRAM bounce +
            # partition-stride-0 load
            q_dram = dramp.tile([BPC, H], F32)
            nc.gpsimd.dma_start(out=q_dram, in_=q_sb)
            q_bc = []
            for b in range(BPC):
                qb_t = qbp.tile([128, H], F32, tag=f"qbc{b}")
                nc.gpsimd.dma_start(
                    out=qb_t, in_=_ap(q_dram.tensor, b * H, [[0, 128], [1, H]])
                )
                q_bc.append(qb_t)

            # ---- main loop over batches ----
            # streaming pools opened after the wq pool release so they can
            # reuse its SBUF space
            with (
                tc.tile_pool(name="pkp", bufs=3) as pkp,
                tc.tile_pool(name="tp", bufs=3) as tp,
                tc.tile_pool(name="vp", bufs=3) as vp,
                tc.tile_pool(name="sm", bufs=2) as sm,
                tc.tile_pool(name="psc", bufs=2, space="PSUM") as psc,
            ):
                pending_ctx = [None] * BPC
                pending_al = [None] * BPC
                for b in range(BPC):
                    # scores phase: [128 s, 16 cols]
                    scores_b = sm.tile([128, SC], F32, tag="scores")
                    for g in range(SC // PKG):
                        pk_t = pkp.tile([128, PKG, H], F32)
                        nc.gpsimd.dma_start(
                            out=pk_t,
                            in_=_ap(
                                pk,
                                b * S * H + g * PKG * 128 * H,
                                [[H, 128], [128 * H, PKG], [1, H]],
                            ),
                        )
                        for cc in range(PKG):
                            c = g * PKG + cc
                            t_t = tp.tile([128, H], F32)
                            nc.vector.tensor_add(out=t_t, in0=pk_t[:, cc, :], in1=q_bc[b])
                            nc.scalar.activation(
                                out=t_t, in_=t_t, func=mybir.ActivationFunctionType.Tanh
                            )
                            nc.vector.tensor_mul(out=t_t, in0=t_t, in1=ve_bcast)
                            # weighted reduce over h: Identity activation with
                            # free-dim accumulator (keeps the reduce off DVE)
                            nc.scalar.activation(
                                out=t_t,
                                in_=t_t,
                                func=mybir.ActivationFunctionType.Identity,
                                accum_out=scores_b[:, c : c + 1],
                            )

                    # masked softmax (no max subtraction; scores bounded by ~26)
                    mask_i = sm.tile([128, SC], I32, tag="mask_i")
                    nc.gpsimd.dma_start(
                        out=mask_i, in_=_ap(mask, b * S, [[1, 128], [128, SC]])
                    )
                    mask_f = sm.tile([128, SC], F32, tag="mask_f")
                    nc.vector.tensor_copy(out=mask_f, in_=mask_i)

                    e_t = sm.tile([128, SC], F32, tag="e")
                    nc.scalar.activation(
                        out=e_t, in_=scores_b, func=mybir.ActivationFunctionType.Exp
                    )
                    nc.vector.tensor_mul(out=e_t, in0=e_t, in1=mask_f)
                    rowsum = sm.tile([128, 1], F32, tag="rowsum")
                    nc.vector.reduce_sum(out=rowsum, in_=e_t, axis=mybir.AxisListType.X)

                    # Z broadcast via ones matmul, written into column 0 of
                    # the ctx psum tile (a [128, VS] psum tile costs the same
                    # 4 banks as [1, VS]; sharing it frees the z bank so ctx
                    # can double-buffer in exactly 8 banks). The first ctx
                    # matmul resets partition 0 with start=True after recip
                    # has read the column.
                    ctxp = psc.tile([128, VS], F32, tag="ctx")
                    nc.tensor.matmul(
                        out=ctxp[:, 0:1],
                        lhsT=ones128,
                        rhs=rowsum,
                        start=True,
                        stop=True,
                        skip_group_check=True,
                    )
                    recip = sm.tile([128, 1], F32, tag="recip")
                    nc.vector.tensor_copy(out=recip, in_=ctxp[:, 0:1])
                    nc.vector.reciprocal(out=recip, in_=recip)

                    alphas_t = sm.tile([128, SC], F32, tag="alphas")
                    nc.vector.tensor_scalar_mul(out=alphas_t, in0=e_t, scalar1=recip)
                    # store batch b-1's alphas now (long since computed): an
                    # alphas store for THIS batch would make the Pool
                    # sequencer wait on the softmax, delaying the pk issues
                    # for the next batch that follow it in Pool program order
                    if pending_al[b - 1] is not None:
                        prev_b, prev_al = pending_al[b - 1]
                        nc.gpsimd.dma_start(
                            out=_ap(al_out, prev_b * S, [[1, 128], [128, SC]]),
                            in_=prev_al,
                        )
                        pending_al[b - 1] = None
                    pending_al[b] = (b, alphas_t)

                    # context phase: ctx[v] = sum_s alphas[s] * value[b,s,v].
                    # alphas-stationary: lhsT = alphas column (f32r), rhs = value
                    # s-chunk [128, 512] (f32r via DMA cast), out = psum [1, 512]
                    # per 512-wide bank, accumulated over the 16 s-chunks.
                    if USE_F32R_CTX:
                        alphas_r = sm.tile([128, SC], F32R, tag="alphas_r")
                        nc.vector.tensor_copy(out=alphas_r, in_=alphas_t)
                    else:
                        alphas_r = alphas_t

                    # drain batch b-1's context psum now: its matmuls finished
                    # while this batch's scores were computing, so this DVE
                    # copy does not stall the engine, and with the psum
                    # double-buffered it does not gate batch b's matmuls
                    if pending_ctx[b - 1] is not None:
                        prev_b, prev_ctxp = pending_ctx[b - 1]
                        ctx_sb = sm.tile([1, VS], F32, tag="ctx_sb")
                        nc.vector.tensor_copy(out=ctx_sb, in_=prev_ctxp[0:1, :])
                        nc.gpsimd.dma_start(
                            out=_ap(ctx_out, prev_b * VS, [[VS, 1], [1, VS]]),
                            in_=ctx_sb,
                        )
                        pending_ctx[b - 1] = None
                    for g in range(SC // VG):
                        v_t = vp.tile([128, VG, VS], vdt)
                        # bitcast the DRAM source view to the tile dtype: bytes
                        # pass through unchanged (no SWDGE cast in the datapath),
                        # the f32r rounding happens inside the PE
                        v_src = _ap(
                            value,
                            b * S * VS + g * VG * 128 * VS,
                            [[VS, 128], [128 * VS, VG], [1, VS]],
                        )
                        if USE_F32R_CTX:
                            v_src = v_src.bitcast(F32R)
                        nc.sync.dma_start(out=v_t, in_=v_src)
                        for cc in range(VG):
                            c = g * VG + cc
                            for j in range(VS // 512):
                                nc.tensor.matmul(
                                    out=ctxp[0:1, j * 512 : (j + 1) * 512],
                                    lhsT=alphas_r[:, c : c + 1],
                                    rhs=v_t[:, cc, j * 512 : (j + 1) * 512],
                                    start=(c == 0),
                                    stop=(c == SC - 1),
                                    skip_group_check=True,
                                )
                    pending_ctx[b] = (b, ctxp)

                last_ab, last_al = pending_al[BPC - 1]
                nc.gpsimd.dma_start(
                    out=_ap(al_out, last_ab * S, [[1, 128], [128, SC]]),
                    in_=last_al,
                )
                last_b, last_ctxp = pending_ctx[BPC - 1]
                ctx_sb = sm.tile([1, VS], F32, tag="ctx_sb")
                nc.vector.tensor_copy(out=ctx_sb, in_=last_ctxp[0:1, :])
                nc.gpsimd.dma_start(
                    out=_ap(ctx_out, last_b * VS, [[VS, 1], [1, VS]]),
                    in_=ctx_sb,
                )

    return _split_excess_waits(nc)


_NC_CACHE = None


def _get_nc():
    global _NC_CACHE
    if _NC_CACHE is None:
        _NC_CACHE = build_nc()
    return _NC_CACHE


def kernel(query, proj_key, value, mask, Wq, v_energy, _want_results_obj=False,
           _trace=False):
    query = np.asarray(query, dtype=np.float32)
    proj_key = np.asarray(proj_key, dtype=np.float32)
    value = np.asarray(value, dtype=np.float32)
    mask = np.asarray(mask, dtype=np.int32)
    Wq = np.asarray(Wq, dtype=np.float32)
    v_energy = np.asarray(v_energy, dtype=np.float32)

    nc = _get_nc()
    in_maps = []
    for k in range(NCORES):
        sl = slice(k * BPC, (k + 1) * BPC)
        in_maps.append(
            {
                "query": query[sl],
                "proj_key": proj_key[sl],
                "value": value[sl],
                "mask": mask[sl],
                "Wq": Wq,
                "v_energy": v_energy,
            }
        )
    res = run_bass_kernel_spmd(
        nc, in_maps, core_ids=list(range(NCORES)), trace=_trace
    )
    ctx = np.concatenate([r["context"] for r in res.results], axis=0)
    al = np.concatenate([r["alphas"] for r in res.results], axis=0)
    if _want_results_obj:
        return (ctx, al), res
    return ctx, al

